# revision 30
# baseline (speedup 1.0000x reference)
"""DETR loss kernel for Trainium2 (8 NeuronCores, SPMD data-parallel over batch).

Device (bass/Tile, per core, 8 of the 64 batches):
  streams the pred_logits shard [800, 2048] f32 from HBM once (the memory-bound
  bulk of this loss) and computes sum(exp(logits)) per (b, q) row via the ACT
  engine's Exp with accum_out. Row sums collect into one [128, 8] SBUF tile,
  PE-transposed so the result leaves as a single contiguous DMA.

Host (numpy, tiny O(B*Q*N) / O(B*Q) work, mirroring the reference which also
does the Hungarian assignment host-side): label-column gather, cost matrix,
Jonker-Volgenant matching, weighted CE + L1 reduction to the final scalar.
"""

import sys
from contextlib import ExitStack

import numpy as np

# ---- problem constants (hardcoded per the fixed problem spec) ----
NUM_CLASSES = 2047
C1 = NUM_CLASSES + 1  # 2048 classes incl. no-object class 0
TIME_WEIGHT = 2.0
EOS_COEF = 0.1
B, Q, N = 64, 100, 16
N_CORES = 8
BPC = B // N_CORES  # batches per core
ROWS = BPC * Q      # (b, q) rows per core
P = 128             # SBUF partitions

for _p in ("/opt/trn_rl_repo",):
    if _p not in sys.path:
        sys.path.insert(0, _p)

# (nrows, rows_per_partition) per tile; must sum to ROWS. rpp>1 packs rpp
# consecutive DRAM rows into one partition so each DMA packet is rpp*8KB
# contiguous.
TILES = ((256, 2), (256, 2), (256, 2), (32, 1))

_NC = None
LAST_RESULT = None  # BassKernelResults of the most recent device run


def build_nc_raw(tiles=None, split_rings=False):
    """Raw-bass version: minimal framework overhead (no Tile/Bacc passes).

    sync: all loads FIFO on one HW-DGE ring (in-order completion), final store.
    scalar: per-row-phase Exp with accum_out into columns of se_all [128, 8].
    gpsimd: identity build; tensor: PE transpose se_all -> PSUM [8, 128];
    vector: se_all memset, PSUM -> SBUF copy. One 8-packet output DMA.
    Output sumexpT [8, 128]: row k = k-th exp-slice's per-partition sums.
    """
    import concourse.bass as bass
    from concourse.masks import make_identity
    from concourse import mybir

    tiles = tiles or TILES
    assert sum(t[0] for t in tiles) == ROWS
    nslices = sum(t[1] for t in tiles)
    assert nslices <= 8

    def _tile_split(tt, global_split):
        if len(tt) > 2:
            return tt[2] == "split"
        return global_split and tt[0] // tt[1] > 32

    nc = bass.Bass("TRN2", target_bir_lowering=False, debug=False)
    logits_d = nc.dram_tensor(
        "logits", [ROWS, C1], mybir.dt.float32, kind="ExternalInput"
    )
    out_d = nc.dram_tensor(
        "sumexpT", [8, P], mybir.dt.float32, kind="ExternalOutput"
    )

    with ExitStack() as ctx:
        bufs = []
        for i, tt in enumerate(tiles):
            bufs.append(
                ctx.enter_context(
                    nc.sbuf_tensor(f"t{i}", [P, tt[1] * C1], mybir.dt.float32)
                )
            )
        se_all = ctx.enter_context(nc.sbuf_tensor("se_all", [P, 8], mybir.dt.float32))
        ident = ctx.enter_context(nc.sbuf_tensor("ident", [P, P], mybir.dt.float32))
        seT = ctx.enter_context(nc.sbuf_tensor("seT", [8, P], mybir.dt.float32))
        seT_ps = ctx.enter_context(nc.psum_tensor("seT_ps", [8, P], mybir.dt.float32))
        scratch = ctx.enter_context(nc.sbuf_tensor("scr", [1, 8], mybir.dt.float32))
        dsems = [
            ctx.enter_context(nc.semaphore(f"dsem{i}")) for i in range(len(tiles))
        ]
        bsems = [
            ctx.enter_context(nc.semaphore(f"bsem{i}")) for i in range(len(tiles))
        ] if any(_tile_split(t, split_rings) for t in tiles) else [None] * len(tiles)
        osem = ctx.enter_context(nc.semaphore("osem"))
        asem = ctx.enter_context(nc.semaphore("asem"))
        vsem = ctx.enter_context(nc.semaphore("vsem"))
        gsem = ctx.enter_context(nc.semaphore("gsem"))
        tsem = ctx.enter_context(nc.semaphore("tsem"))
        # const AP (initialized in the preamble) for the dummy table-warm ACT
        dummy_in = nc.const_aps.scalar_like(0.0, scratch[:, :])
        block = ctx.enter_context(nc.Block())

        @block.sync
        def _(sync):
            r0 = 0
            for i, tt in enumerate(tiles):
                nrows, rpp = tt[0], tt[1]
                parts = nrows // rpp
                if _tile_split(tt, split_rings):
                    ha = parts // 2
                    src = logits_d[r0 : r0 + ha * rpp, :].rearrange(
                        "(p r) c -> p (r c)", r=rpp
                    )
                    sync.dma_start(out=bufs[i][:ha, :], in_=src).then_inc(
                        dsems[i], 16
                    )
                else:
                    src = logits_d[r0 : r0 + nrows, :].rearrange(
                        "(p r) c -> p (r c)", r=rpp
                    )
                    sync.dma_start(out=bufs[i][:parts, :], in_=src).then_inc(
                        dsems[i], 16
                    )
                r0 += nrows
            sync.wait_ge(vsem, 2)
            sync.dma_start(out=out_d[:, :], in_=seT[:, :]).then_inc(osem, 16)
            sync.wait_ge(osem, 16)

        @block.scalar
        def _(scalar):
            r0 = 0
            for i, tt in enumerate(tiles):
                nrows, rpp = tt[0], tt[1]
                parts = nrows // rpp
                if _tile_split(tt, split_rings):
                    ha = parts // 2
                    src = logits_d[r0 + ha * rpp : r0 + nrows, :].rearrange(
                        "(p r) c -> p (r c)", r=rpp
                    )
                    scalar.dma_start(
                        out=bufs[i][ha:parts, :], in_=src
                    ).then_inc(bsems[i], 16)
                r0 += nrows
            # dummy tiny exp so walrus's ACT table load runs before any waits
            nc.scalar.activation(
                out=scratch[:, :1], in_=dummy_in,
                func=mybir.ActivationFunctionType.Exp,
            )
            scalar.wait_ge(vsem, 1)
            col = 0
            for i, tt in enumerate(tiles):
                nrows, rpp = tt[0], tt[1]
                parts = nrows // rpp
                scalar.wait_ge(dsems[i], 16)
                if _tile_split(tt, split_rings):
                    scalar.wait_ge(bsems[i], 16)
                for j in range(rpp):
                    nc.scalar.activation(
                        out=bufs[i][:parts, j * C1 : (j + 1) * C1],
                        in_=bufs[i][:parts, j * C1 : (j + 1) * C1],
                        func=mybir.ActivationFunctionType.Exp,
                        accum_out=se_all[:parts, col : col + 1],
                    ).then_inc(asem, 1)
                    col += 1

        @block.gpsimd
        def _(gpsimd):
            nc.gpsimd.memset(ident[:, :], 0.0)
            nc.gpsimd.drain()
            make_identity(nc, ident[:, :], nomemset=True)
            nc.gpsimd.drain()
            nc.gpsimd.engine_nop().then_inc(gsem, 1)

        @block.tensor
        def _(tensor):
            tensor.wait_ge(gsem, 1)
            tensor.wait_ge(asem, nslices)
            nc.tensor.transpose(
                out=seT_ps[:, :], in_=se_all[:, :], identity=ident[:, :]
            ).then_inc(tsem, 1)

        @block.vector
        def _(vector):
            nc.vector.memset(se_all[:, :], 0.0).then_inc(vsem, 1)
            vector.wait_ge(tsem, 1)
            nc.vector.tensor_copy(seT[:, :], seT_ps[:, :]).then_inc(vsem, 1)

    return nc


def unpack_sumexp(out_t, tiles=None):
    """out_t: [P, P] from one core -> per-row sumexp [ROWS]."""
    tiles = tiles or TILES
    se = np.empty(ROWS, np.float32)
    col = 0
    r0 = 0
    for tt in tiles:
        nrows, rpp = tt[0], tt[1]
        parts = nrows // rpp
        # slice j, partition p -> row r0 + p*rpp + j
        blk = out_t[col : col + rpp, :parts]  # [rpp, parts]
        se[r0 : r0 + nrows] = blk.T.reshape(nrows)
        col += rpp
        r0 += nrows
    return se


_IDENT = np.eye(P, dtype=np.float32)


def make_in_maps(pl, with_ident=False):
    """pl: [B, Q, C1] f32 -> per-core input dicts."""
    maps = [
        {
            "logits": np.ascontiguousarray(
                pl[k * BPC : (k + 1) * BPC].reshape(ROWS, C1)
            ),
        }
        for k in range(N_CORES)
    ]
    if with_ident:
        for m in maps:
            m["ident"] = _IDENT
    return maps


def unpack_sumexp_raw(out, tiles=None):
    """out: [8, 128] raw-kernel output (seT) -> per-row sumexp [ROWS]."""
    return unpack_sumexp(out, tiles=tiles)


def _ensure_trace_plumbing():
    """Make run_bass_kernel_spmd's optional NTFF-trace path (env BASS_TRACE)
    functional on this image: provide antenv.axon_hooks if the image's antenv
    lacks it, and keep profile artifacts local if the upload backend is
    unreachable. No-ops when tracing stays off."""
    import types

    try:
        import antenv.axon_hooks  # noqa: F401
    except ImportError:
        try:
            import antenv
        except ImportError:
            return
        mod = types.ModuleType("antenv.axon_hooks")
        holder = [None]
        mod.set_axon_ntff_profile_hook = lambda h: holder.__setitem__(0, h)
        mod.get_axon_ntff_profile_hook = lambda: holder[0]
        sys.modules["antenv.axon_hooks"] = mod
        antenv.axon_hooks = mod
        try:
            from trn_agent_boot.trn_boot import _ntff_profile_via_ctypes

            mod.set_axon_ntff_profile_hook(
                _ntff_profile_via_ctypes("/opt/axon/libaxon_pjrt.so")
            )
        except Exception:
            pass
    try:
        from concourse import bass_utils

        orig = bass_utils.upload_artifacts

        def _safe_upload(tmpdir):
            try:
                return orig(tmpdir)
            except Exception:
                return f"file://{tmpdir}"

        if getattr(bass_utils.upload_artifacts, "__name__", "") != "_safe_upload":
            bass_utils.upload_artifacts = _safe_upload
    except Exception:
        pass


def _run_device(pl):
    _ensure_trace_plumbing()
    from concourse.bass_utils import run_bass_kernel_spmd

    global _NC, LAST_RESULT
    if _NC is None:
        _NC = build_nc_raw()
    res = run_bass_kernel_spmd(_NC, make_in_maps(pl), core_ids=list(range(N_CORES)))
    LAST_RESULT = res
    sumexp = np.stack(
        [unpack_sumexp_raw(r["sumexpT"]) for r in res.results]
    ).reshape(B, Q)
    return sumexp


def _lsa(cost):
    """Jonker-Volgenant min-cost assignment for rectangular cost [R, C], R <= C.
    Returns (row_ind, col_ind) with row_ind = arange(R). Equivalent to
    scipy.optimize.linear_sum_assignment."""
    R, Cn = cost.shape
    u = np.zeros(R + 1)
    v = np.zeros(Cn + 1)
    p = np.zeros(Cn + 1, dtype=np.int64)  # p[j] = 1-based row matched to col j (0 = free)
    for i in range(1, R + 1):
        p[0] = i
        j0 = 0
        minv = np.full(Cn + 1, np.inf)
        used = np.zeros(Cn + 1, dtype=bool)
        way = np.zeros(Cn + 1, dtype=np.int64)
        while True:
            used[j0] = True
            i0 = p[j0]
            cur = cost[i0 - 1] - u[i0] - v[1:]
            free = ~used[1:]
            upd = free & (cur < minv[1:])
            minv[1:] = np.where(upd, cur, minv[1:])
            way[1:] = np.where(upd, j0, way[1:])
            j1 = int(np.argmin(np.where(free, minv[1:], np.inf))) + 1
            delta = minv[j1]
            u[p[used]] += delta
            v[used] -= delta
            minv[1:] = np.where(free, minv[1:] - delta, minv[1:])
            j0 = j1
            if p[j0] == 0:
                break
        while j0 != 0:  # augment along the alternating path
            j1 = way[j0]
            p[j0] = p[j1]
            j0 = j1
    col4row = np.zeros(R, dtype=np.int64)
    for j in range(1, Cn + 1):
        if p[j] > 0:
            col4row[p[j] - 1] = j - 1
    return np.arange(R), col4row


def _host_loss(pl, pt, lb64, tstamp, sumexp):
    logZ = np.log(sumexp.astype(np.float64))  # [B, Q]

    # gather the 16 label-class logits + the class-0 logit per (b, q)
    cols = np.concatenate([lb64, np.zeros((B, 1), np.int64)], axis=1)  # [B, N+1]
    glog = np.take_along_axis(
        pl.astype(np.float64), cols[:, None, :].repeat(Q, axis=1), axis=2
    )  # [B, Q, N+1]

    # --- Hungarian matching cost (as in the reference) ---
    cost_class = -np.exp(glog[:, :, :N] - logZ[:, :, None])  # [B, Q, N]
    cost_time = np.abs(
        pt[:, :, 0][:, :, None].astype(np.float64)
        - tstamp[:, :, 0][:, None, :].astype(np.float64)
    )
    Cmat = cost_class + TIME_WEIGHT * cost_time
    Cmat = np.nan_to_num(Cmat, nan=100.0, posinf=100.0, neginf=-100.0)

    qs = np.zeros((B, N), np.int64)
    tsi = np.zeros((B, N), np.int64)
    for i in range(B):
        r, c = _lsa(Cmat[i].T)  # rows = targets (N), cols = queries (Q)
        tsi[i] = r
        qs[i] = c

    # --- weighted cross entropy over all queries ---
    batch_ar = np.arange(B)[:, None]
    class_ids = np.take_along_axis(lb64, tsi, axis=1)  # [B, N]
    target_classes = np.zeros((B, Q), np.int64)
    target_classes[batch_ar, qs] = class_ids
    tgt_logit = glog[:, :, N].copy()  # default: class-0 logit
    tgt_logit[batch_ar, qs] = glog[batch_ar, qs, tsi]
    nll = logZ - tgt_logit
    w = np.where(target_classes == 0, EOS_COEF, 1.0)
    loss_ce = (w * nll).sum() / w.sum()

    # --- L1 loss on matched timestamps ---
    matched_pred = pt[batch_ar, qs, 0].astype(np.float64)
    matched_tgt = np.take_along_axis(tstamp[:, :, 0].astype(np.float64), tsi, axis=1)
    loss_time = np.abs(matched_pred - matched_tgt).mean()

    return np.asarray(loss_ce + TIME_WEIGHT * loss_time, dtype=np.float32)


def kernel(pred_logits, pred_time, labels, timestamps):
    pl = np.asarray(pred_logits, dtype=np.float32)
    pt = np.asarray(pred_time, dtype=np.float32)
    lb64 = np.asarray(labels).astype(np.int64)
    tstamp = np.asarray(timestamps, dtype=np.float32)

    sumexp = _run_device(pl)
    return _host_loss(pl, pt, lb64, tstamp, sumexp)


# revision 31
# speedup vs baseline: 1.1325x; 1.1325x over previous
"""DETR loss kernel for Trainium2 (8 NeuronCores, SPMD data-parallel over batch).

Device (bass/Tile, per core, 8 of the 64 batches):
  streams the pred_logits shard [800, 2048] f32 from HBM once (the memory-bound
  bulk of this loss) and computes sum(exp(logits)) per (b, q) row via the ACT
  engine's Exp with accum_out. Row sums collect into one [128, 8] SBUF tile,
  PE-transposed so the result leaves as a single contiguous DMA.

Host (numpy, tiny O(B*Q*N) / O(B*Q) work, mirroring the reference which also
does the Hungarian assignment host-side): label-column gather, cost matrix,
Jonker-Volgenant matching, weighted CE + L1 reduction to the final scalar.
"""

import sys
from contextlib import ExitStack

import numpy as np

# ---- problem constants (hardcoded per the fixed problem spec) ----
NUM_CLASSES = 2047
C1 = NUM_CLASSES + 1  # 2048 classes incl. no-object class 0
TIME_WEIGHT = 2.0
EOS_COEF = 0.1
B, Q, N = 64, 100, 16
N_CORES = 8
BPC = B // N_CORES  # batches per core
ROWS = BPC * Q      # (b, q) rows per core
P = 128             # SBUF partitions

for _p in ("/opt/trn_rl_repo",):
    if _p not in sys.path:
        sys.path.insert(0, _p)

# (nrows, rows_per_partition) per tile; must sum to ROWS. rpp>1 packs rpp
# consecutive DRAM rows into one partition so each DMA packet is rpp*8KB
# contiguous.
# small tile first: warms the ACT chain early and leaves only a two-slice
# batch after the final DMA -> shorter exposed exp tail. All packets 16KB.
TILES = ((32, 2), (256, 2), (256, 2), (256, 2))

_NC = None
LAST_RESULT = None  # BassKernelResults of the most recent device run


def build_nc_raw(tiles=None, split_rings=False):
    """Raw-bass version: minimal framework overhead (no Tile/Bacc passes).

    sync: all loads FIFO on one HW-DGE ring (in-order completion), final store.
    scalar: per-row-phase Exp with accum_out into columns of se_all [128, 8].
    gpsimd: identity build; tensor: PE transpose se_all -> PSUM [8, 128];
    vector: se_all memset, PSUM -> SBUF copy. One 8-packet output DMA.
    Output sumexpT [8, 128]: row k = k-th exp-slice's per-partition sums.
    """
    import concourse.bass as bass
    from concourse.masks import make_identity
    from concourse import mybir

    tiles = tiles or TILES
    assert sum(t[0] for t in tiles) == ROWS
    nslices = sum(t[1] for t in tiles)
    assert nslices <= 8

    def _tile_split(tt, global_split):
        if len(tt) > 2:
            return tt[2] == "split"
        return global_split and tt[0] // tt[1] > 32

    nc = bass.Bass("TRN2", target_bir_lowering=False, debug=False)
    logits_d = nc.dram_tensor(
        "logits", [ROWS, C1], mybir.dt.float32, kind="ExternalInput"
    )
    out_d = nc.dram_tensor(
        "sumexpT", [8, P], mybir.dt.float32, kind="ExternalOutput"
    )

    with ExitStack() as ctx:
        bufs = []
        for i, tt in enumerate(tiles):
            bufs.append(
                ctx.enter_context(
                    nc.sbuf_tensor(f"t{i}", [P, tt[1] * C1], mybir.dt.float32)
                )
            )
        se_all = ctx.enter_context(nc.sbuf_tensor("se_all", [P, 8], mybir.dt.float32))
        ident = ctx.enter_context(nc.sbuf_tensor("ident", [P, P], mybir.dt.float32))
        seT = ctx.enter_context(nc.sbuf_tensor("seT", [8, P], mybir.dt.float32))
        seT_ps = ctx.enter_context(nc.psum_tensor("seT_ps", [8, P], mybir.dt.float32))
        scratch = ctx.enter_context(nc.sbuf_tensor("scr", [1, 8], mybir.dt.float32))
        dsems = [
            ctx.enter_context(nc.semaphore(f"dsem{i}")) for i in range(len(tiles))
        ]
        bsems = [
            ctx.enter_context(nc.semaphore(f"bsem{i}")) for i in range(len(tiles))
        ] if any(_tile_split(t, split_rings) for t in tiles) else [None] * len(tiles)
        osem = ctx.enter_context(nc.semaphore("osem"))
        asem = ctx.enter_context(nc.semaphore("asem"))
        vsem = ctx.enter_context(nc.semaphore("vsem"))
        gsem = ctx.enter_context(nc.semaphore("gsem"))
        tsem = ctx.enter_context(nc.semaphore("tsem"))
        # const AP (initialized in the preamble) for the dummy table-warm ACT
        dummy_in = nc.const_aps.scalar_like(0.0, scratch[:, :])
        block = ctx.enter_context(nc.Block())

        @block.sync
        def _(sync):
            r0 = 0
            for i, tt in enumerate(tiles):
                nrows, rpp = tt[0], tt[1]
                parts = nrows // rpp
                if _tile_split(tt, split_rings):
                    ha = parts // 2
                    src = logits_d[r0 : r0 + ha * rpp, :].rearrange(
                        "(p r) c -> p (r c)", r=rpp
                    )
                    sync.dma_start(out=bufs[i][:ha, :], in_=src).then_inc(
                        dsems[i], 16
                    )
                else:
                    src = logits_d[r0 : r0 + nrows, :].rearrange(
                        "(p r) c -> p (r c)", r=rpp
                    )
                    sync.dma_start(out=bufs[i][:parts, :], in_=src).then_inc(
                        dsems[i], 16
                    )
                r0 += nrows
            sync.wait_ge(vsem, 2)
            sync.dma_start(out=out_d[:, :], in_=seT[:, :]).then_inc(osem, 16)
            sync.wait_ge(osem, 16)

        @block.scalar
        def _(scalar):
            r0 = 0
            for i, tt in enumerate(tiles):
                nrows, rpp = tt[0], tt[1]
                parts = nrows // rpp
                if _tile_split(tt, split_rings):
                    ha = parts // 2
                    src = logits_d[r0 + ha * rpp : r0 + nrows, :].rearrange(
                        "(p r) c -> p (r c)", r=rpp
                    )
                    scalar.dma_start(
                        out=bufs[i][ha:parts, :], in_=src
                    ).then_inc(bsems[i], 16)
                r0 += nrows
            # dummy tiny exp so walrus's ACT table load runs before any waits
            nc.scalar.activation(
                out=scratch[:, :1], in_=dummy_in,
                func=mybir.ActivationFunctionType.Exp,
            )
            scalar.wait_ge(vsem, 1)
            col = 0
            for i, tt in enumerate(tiles):
                nrows, rpp = tt[0], tt[1]
                parts = nrows // rpp
                scalar.wait_ge(dsems[i], 16)
                if _tile_split(tt, split_rings):
                    scalar.wait_ge(bsems[i], 16)
                for j in range(rpp):
                    nc.scalar.activation(
                        out=bufs[i][:parts, j * C1 : (j + 1) * C1],
                        in_=bufs[i][:parts, j * C1 : (j + 1) * C1],
                        func=mybir.ActivationFunctionType.Exp,
                        accum_out=se_all[:parts, col : col + 1],
                    ).then_inc(asem, 1)
                    col += 1

        @block.gpsimd
        def _(gpsimd):
            nc.gpsimd.memset(ident[:, :], 0.0)
            nc.gpsimd.drain()
            make_identity(nc, ident[:, :], nomemset=True)
            nc.gpsimd.drain()
            nc.gpsimd.engine_nop().then_inc(gsem, 1)

        @block.tensor
        def _(tensor):
            tensor.wait_ge(gsem, 1)
            tensor.wait_ge(asem, nslices)
            nc.tensor.transpose(
                out=seT_ps[:, :], in_=se_all[:, :], identity=ident[:, :]
            ).then_inc(tsem, 1)

        @block.vector
        def _(vector):
            nc.vector.memset(se_all[:, :], 0.0).then_inc(vsem, 1)
            vector.wait_ge(tsem, 1)
            nc.vector.tensor_copy(seT[:, :], seT_ps[:, :]).then_inc(vsem, 1)

    return nc


def unpack_sumexp(out_t, tiles=None):
    """out_t: [P, P] from one core -> per-row sumexp [ROWS]."""
    tiles = tiles or TILES
    se = np.empty(ROWS, np.float32)
    col = 0
    r0 = 0
    for tt in tiles:
        nrows, rpp = tt[0], tt[1]
        parts = nrows // rpp
        # slice j, partition p -> row r0 + p*rpp + j
        blk = out_t[col : col + rpp, :parts]  # [rpp, parts]
        se[r0 : r0 + nrows] = blk.T.reshape(nrows)
        col += rpp
        r0 += nrows
    return se


_IDENT = np.eye(P, dtype=np.float32)


def make_in_maps(pl, with_ident=False):
    """pl: [B, Q, C1] f32 -> per-core input dicts."""
    maps = [
        {
            "logits": np.ascontiguousarray(
                pl[k * BPC : (k + 1) * BPC].reshape(ROWS, C1)
            ),
        }
        for k in range(N_CORES)
    ]
    if with_ident:
        for m in maps:
            m["ident"] = _IDENT
    return maps


def unpack_sumexp_raw(out, tiles=None):
    """out: [8, 128] raw-kernel output (seT) -> per-row sumexp [ROWS]."""
    return unpack_sumexp(out, tiles=tiles)


def _ensure_trace_plumbing():
    """Make run_bass_kernel_spmd's optional NTFF-trace path (env BASS_TRACE)
    functional on this image: provide antenv.axon_hooks if the image's antenv
    lacks it, and keep profile artifacts local if the upload backend is
    unreachable. No-ops when tracing stays off."""
    import types

    try:
        import antenv.axon_hooks  # noqa: F401
    except ImportError:
        try:
            import antenv
        except ImportError:
            return
        mod = types.ModuleType("antenv.axon_hooks")
        holder = [None]
        mod.set_axon_ntff_profile_hook = lambda h: holder.__setitem__(0, h)
        mod.get_axon_ntff_profile_hook = lambda: holder[0]
        sys.modules["antenv.axon_hooks"] = mod
        antenv.axon_hooks = mod
        try:
            from trn_agent_boot.trn_boot import _ntff_profile_via_ctypes

            mod.set_axon_ntff_profile_hook(
                _ntff_profile_via_ctypes("/opt/axon/libaxon_pjrt.so")
            )
        except Exception:
            pass
    try:
        from concourse import bass_utils

        orig = bass_utils.upload_artifacts

        def _safe_upload(tmpdir):
            try:
                return orig(tmpdir)
            except Exception:
                return f"file://{tmpdir}"

        if getattr(bass_utils.upload_artifacts, "__name__", "") != "_safe_upload":
            bass_utils.upload_artifacts = _safe_upload
    except Exception:
        pass


def _run_device(pl):
    _ensure_trace_plumbing()
    from concourse.bass_utils import run_bass_kernel_spmd

    global _NC, LAST_RESULT
    if _NC is None:
        _NC = build_nc_raw()
    res = run_bass_kernel_spmd(_NC, make_in_maps(pl), core_ids=list(range(N_CORES)))
    LAST_RESULT = res
    sumexp = np.stack(
        [unpack_sumexp_raw(r["sumexpT"]) for r in res.results]
    ).reshape(B, Q)
    return sumexp


def _lsa(cost):
    """Jonker-Volgenant min-cost assignment for rectangular cost [R, C], R <= C.
    Returns (row_ind, col_ind) with row_ind = arange(R). Equivalent to
    scipy.optimize.linear_sum_assignment."""
    R, Cn = cost.shape
    u = np.zeros(R + 1)
    v = np.zeros(Cn + 1)
    p = np.zeros(Cn + 1, dtype=np.int64)  # p[j] = 1-based row matched to col j (0 = free)
    for i in range(1, R + 1):
        p[0] = i
        j0 = 0
        minv = np.full(Cn + 1, np.inf)
        used = np.zeros(Cn + 1, dtype=bool)
        way = np.zeros(Cn + 1, dtype=np.int64)
        while True:
            used[j0] = True
            i0 = p[j0]
            cur = cost[i0 - 1] - u[i0] - v[1:]
            free = ~used[1:]
            upd = free & (cur < minv[1:])
            minv[1:] = np.where(upd, cur, minv[1:])
            way[1:] = np.where(upd, j0, way[1:])
            j1 = int(np.argmin(np.where(free, minv[1:], np.inf))) + 1
            delta = minv[j1]
            u[p[used]] += delta
            v[used] -= delta
            minv[1:] = np.where(free, minv[1:] - delta, minv[1:])
            j0 = j1
            if p[j0] == 0:
                break
        while j0 != 0:  # augment along the alternating path
            j1 = way[j0]
            p[j0] = p[j1]
            j0 = j1
    col4row = np.zeros(R, dtype=np.int64)
    for j in range(1, Cn + 1):
        if p[j] > 0:
            col4row[p[j] - 1] = j - 1
    return np.arange(R), col4row


def _host_loss(pl, pt, lb64, tstamp, sumexp):
    logZ = np.log(sumexp.astype(np.float64))  # [B, Q]

    # gather the 16 label-class logits + the class-0 logit per (b, q)
    cols = np.concatenate([lb64, np.zeros((B, 1), np.int64)], axis=1)  # [B, N+1]
    glog = np.take_along_axis(
        pl.astype(np.float64), cols[:, None, :].repeat(Q, axis=1), axis=2
    )  # [B, Q, N+1]

    # --- Hungarian matching cost (as in the reference) ---
    cost_class = -np.exp(glog[:, :, :N] - logZ[:, :, None])  # [B, Q, N]
    cost_time = np.abs(
        pt[:, :, 0][:, :, None].astype(np.float64)
        - tstamp[:, :, 0][:, None, :].astype(np.float64)
    )
    Cmat = cost_class + TIME_WEIGHT * cost_time
    Cmat = np.nan_to_num(Cmat, nan=100.0, posinf=100.0, neginf=-100.0)

    qs = np.zeros((B, N), np.int64)
    tsi = np.zeros((B, N), np.int64)
    for i in range(B):
        r, c = _lsa(Cmat[i].T)  # rows = targets (N), cols = queries (Q)
        tsi[i] = r
        qs[i] = c

    # --- weighted cross entropy over all queries ---
    batch_ar = np.arange(B)[:, None]
    class_ids = np.take_along_axis(lb64, tsi, axis=1)  # [B, N]
    target_classes = np.zeros((B, Q), np.int64)
    target_classes[batch_ar, qs] = class_ids
    tgt_logit = glog[:, :, N].copy()  # default: class-0 logit
    tgt_logit[batch_ar, qs] = glog[batch_ar, qs, tsi]
    nll = logZ - tgt_logit
    w = np.where(target_classes == 0, EOS_COEF, 1.0)
    loss_ce = (w * nll).sum() / w.sum()

    # --- L1 loss on matched timestamps ---
    matched_pred = pt[batch_ar, qs, 0].astype(np.float64)
    matched_tgt = np.take_along_axis(tstamp[:, :, 0].astype(np.float64), tsi, axis=1)
    loss_time = np.abs(matched_pred - matched_tgt).mean()

    return np.asarray(loss_ce + TIME_WEIGHT * loss_time, dtype=np.float32)


def kernel(pred_logits, pred_time, labels, timestamps):
    pl = np.asarray(pred_logits, dtype=np.float32)
    pt = np.asarray(pred_time, dtype=np.float32)
    lb64 = np.asarray(labels).astype(np.int64)
    tstamp = np.asarray(timestamps, dtype=np.float32)

    sumexp = _run_device(pl)
    return _host_loss(pl, pt, lb64, tstamp, sumexp)
